# revision 2
# baseline (speedup 1.0000x reference)
"""Distance-aware label smoothing loss on 8 Trainium2 NeuronCores — v3.

Math: row c of the smoothing matrix M sums to 1, so
    loss_i = logsumexp(logits_i) - smooth_dot_i - conf * logits[i, t_i]
The off-diagonal smoothing dot splits into a uniform part (device rowsum)
plus an exact near-diagonal band correction done on host (O(B*W) numpy),
leaving a ~3e-7 residual (see kernel2 docstring for the derivation).

Device kernel (per core, 2048 rows = 16 tiles of [128, 1000]):
  - host casts logits to bf16 (round-to-nearest): stream is 4.1 MB instead
    of 8.2 MB — the kernel is DMA-bound, so this halves its runtime. The
    host-side band/diag corrections still use the exact f32 logits.
  - exp is computed on DVE via the Schraudolph bit trick in bf16 domain:
      i16 = int16(a*x + b),  a = 128/ln2,  b = 128*(127 - 0.05535)
    whose bits, reinterpreted as bf16, approximate exp(x). The shift
    0.05535 centers the log-domain sawtooth error (analytic constant,
    robust to round- or truncate-on-convert); final loss lands ~2e-4 rel.
    The same instruction's f32 accum_out yields a*rowsum + C*b for free.
  - pass 2: tensor_scalar copy of the bits tile bitcast to bf16, f32
    accum_out -> sumexp. Both passes are 2-byte packed all-SBUF
    tensor_scalar ops, eligible for the DVE 4x perf mode (~330 ns/tile
    vs ~1.2 us for an ACT exp), so compute hides entirely under the DMA.
  - ACT computes lse = Ln(sumexp); out DMA [128, 32] = (lse | raw accum).
Host: rowsum = (accum - C*b)/a, then band + diag corrections in f64.
"""

import numpy as np

import concourse.bass as bass
import concourse.tile as tile
from concourse import mybir
from concourse.bass_utils import run_bass_kernel_spmd

N_CORES = 8
B, C = 16384, 1000
ROWS = B // N_CORES  # 2048 rows per core
P = 128
NTILES = ROWS // P  # 16
SMOOTHING = 0.1
CONFIDENCE = 1.0 - SMOOTHING
W_BAND = 32

EXP_A = 128.0 / np.log(2.0)
EXP_B = 128.0 * (127.0 - 0.05535)

BATCH = 2  # tiles per DMA instruction
BUFS = 8

F32 = mybir.dt.float32
BF16 = mybir.dt.bfloat16
I16 = mybir.dt.int16

_NC_CACHE = {}
_HOST_CACHE = {}


def _build_nc(reps=1, batch=BATCH, bufs=BUFS, rings=("sync",)):
    """reps>1 wraps the body in a device For_i loop (timing runs only)."""
    assert NTILES % batch == 0
    ngroups = NTILES // batch
    nc = bass.Bass()

    # host pre-tiles [ROWS, C] -> [P, NTILES*C] so each partition's stream
    # is contiguous in DRAM (one descriptor per partition per load)
    logits_in = nc.dram_tensor("logits", [P, NTILES * C], BF16, kind="ExternalInput")
    out_t = nc.dram_tensor("out", [P, 2 * NTILES], F32, kind="ExternalOutput")

    ring_engines = [getattr(nc, r) for r in rings]

    with tile.TileContext(nc) as tc:
        with (
            tc.tile_pool(name="lts", bufs=bufs) as lts,
            tc.tile_pool(name="bits", bufs=4) as bitsp,
            tc.tile_pool(name="dead", bufs=2) as deadp,
            tc.tile_pool(name="stats", bufs=1) as stats,
        ):
            sumexp = stats.tile([P, NTILES], F32)
            outst = stats.tile([P, 2 * NTILES], F32)

            def emit_group(g):
                lt = lts.tile([P, batch * C], BF16, tag="lt")
                cols = slice(g * batch * C, (g + 1) * batch * C)
                eng = ring_engines[g % len(ring_engines)]
                eng.dma_start(out=lt[:, :], in_=logits_in[:, cols])
                for a in range(batch):
                    j = g * batch + a
                    cs = slice(a * C, (a + 1) * C)
                    bt = bitsp.tile([P, C], I16, tag="bt")
                    # pass 1: exp bits + (a*rowsum + C*b) accumulation.
                    # host ships bf16(a*x); op0 adds b, op1 is the REDUCE
                    # operator of the accum (TensorScalarPtrReduce), and the
                    # accum sums the pre-conversion f32 values.
                    nc.vector.tensor_scalar(
                        out=bt[:, :],
                        in0=lt[:, cs],
                        scalar1=float(EXP_B),
                        scalar2=0.0,
                        op0=mybir.AluOpType.add,
                        op1=mybir.AluOpType.add,
                        accum_out=outst[:, NTILES + j : NTILES + j + 1],
                    )
                    # pass 2: sumexp of the bits reinterpreted as bf16
                    dt_ = deadp.tile([P, C], BF16, tag="dt")
                    nc.vector.tensor_scalar(
                        out=dt_[:, :],
                        in0=bt[:, :].bitcast(BF16),
                        scalar1=1.0,
                        scalar2=0.0,
                        op0=mybir.AluOpType.mult,
                        op1=mybir.AluOpType.add,
                        accum_out=sumexp[:, j : j + 1],
                    )
                # per-group Ln overlaps the tail with later groups' work
                js = slice(g * batch, (g + 1) * batch)
                nc.scalar.activation(
                    out=outst[:, js],
                    in_=sumexp[:, js],
                    func=mybir.ActivationFunctionType.Ln,
                )

            if reps == 1:
                for g in range(ngroups):
                    emit_group(g)
            else:
                with tc.For_i(0, reps, 1):
                    for g in range(ngroups):
                        emit_group(g)

            nc.sync.dma_start(out=out_t[:, :], in_=outst[:, :])

    return _split_sync_waits(nc)


_WAIT_LIMIT = 1


def _split_sync_waits(nc, limit=_WAIT_LIMIT):
    """Walrus ISA structs have few sync-wait slots; Tile can emit more.

    Move excess waits onto same-engine InstNoOp fillers placed right before
    the over-subscribed instruction (engine stalls on them in order, so the
    blocking semantics are unchanged)."""
    idx = 0
    for fn in nc.m.functions:
        for b in fn.blocks:
            out = []
            for inst in b.instructions:
                si = inst.sync_info
                waits = list(si.on_wait) if (si is not None and si.on_wait) else []
                if len(waits) > limit:
                    excess, keep = waits[:-limit], waits[-limit:]
                    for k in range(0, len(excess), limit):
                        nop = mybir.InstNoOp(
                            name=f"waitsplit_{idx}", ins=[], outs=[]
                        )
                        idx += 1
                        nop.engine = inst.engine
                        nop.sync_info = mybir.SyncInfo(
                            on_wait=excess[k : k + limit], on_update=[]
                        )
                        out.append(nop)
                    inst.sync_info = mybir.SyncInfo(
                        on_wait=keep, on_update=list(si.on_update)
                    )
                out.append(inst)
            b.instructions = out
    return nc


def _tile_layout(a):
    """[ROWS, C] -> [P, NTILES*C]: partition p holds rows j*P+p for all j."""
    return np.ascontiguousarray(
        a.reshape(NTILES, P, C).transpose(1, 0, 2).reshape(P, NTILES * C)
    )


def build_in_maps(logits, t):
    # pre-scale by a so the device only needs one add before the int16
    # convert; bf16 relative precision is scale-invariant
    xb = (EXP_A * logits).astype(mybir.dt.np(BF16))
    in_maps = []
    for k in range(N_CORES):
        rows = slice(k * ROWS, (k + 1) * ROWS)
        in_maps.append({"logits": _tile_layout(xb[rows])})
    return in_maps


def _host_static():
    """Per-class off-diagonal normalizer total_c = sum_{k != c} 1/(|k-c|+1)."""
    if "tot" not in _HOST_CACHE:
        dist = np.abs(
            np.arange(C)[:, None] - np.arange(C)[None, :]
        ).astype(np.float64)
        w = 1.0 / (dist + 1.0)
        np.fill_diagonal(w, 0.0)
        _HOST_CACHE["tot"] = w.sum(1)
    return _HOST_CACHE["tot"]


def _host_correction(logits, t, lse, rowsum):
    """Combine device per-row (lse, rowsum) with the exact band + diag terms."""
    tot = _host_static()
    lg = logits.astype(np.float64)
    t = t.astype(np.int64)
    ki = np.arange(-W_BAND, W_BAND + 1)
    cols = t[:, None] + ki[None, :]
    valid = (cols >= 0) & (cols < C)
    colsc = np.clip(cols, 0, C - 1)
    lg_band = np.take_along_axis(lg, colsc, axis=1) * valid
    m_band = (SMOOTHING / tot[t])[:, None] * (1.0 / (np.abs(ki)[None, :] + 1.0))
    m_band[:, W_BAND] = 0.0  # k = t carries confidence, not smoothing
    m_band = m_band * valid
    u = (SMOOTHING - m_band.sum(1)) / (C - valid.sum(1))
    diag = lg[np.arange(lg.shape[0]), t]
    dot = u * (rowsum.astype(np.float64) - lg_band.sum(1)) + (
        m_band * lg_band
    ).sum(1)
    loss = lse.astype(np.float64) - dot - CONFIDENCE * diag
    return loss.mean()


def kernel(logits, targets):
    logits = np.ascontiguousarray(np.asarray(logits), dtype=np.float32)
    t = np.asarray(targets).astype(np.int64).ravel()
    assert logits.shape == (B, C) and t.shape == (B,)

    if "nc" not in _NC_CACHE:
        _NC_CACHE["nc"] = _build_nc()
    nc = _NC_CACHE["nc"]

    in_maps = build_in_maps(logits, t)
    res = run_bass_kernel_spmd(nc, in_maps, core_ids=list(range(N_CORES)))

    lse = np.empty(B, np.float64)
    rowsum = np.empty(B, np.float64)
    for k, r in enumerate(res.results):
        o = r["out"].astype(np.float64)  # [P, 2*NTILES]
        rows = slice(k * ROWS, (k + 1) * ROWS)
        # local row r = j*P + p  <->  [p, j]
        lse[rows] = o[:, :NTILES].T.ravel()
        rowsum[rows] = (o[:, NTILES:].T.ravel() - C * EXP_B) / EXP_A

    return np.asarray(np.float32(_host_correction(logits, t, lse, rowsum)))
